# revision 1
# baseline (speedup 1.0000x reference)
"""DiskLoss Trainium2 kernel.

Computes the reference loss:
  pred = gather(output, ind)            # [K,33] per batch
  gt_m = even-odd rasterization of the 16-gon from target   (per object)
  dk_m = union of 15 disks (radius ceil(|pred[:,32]|)) from pred
  per_obj = 1 - inter/(union+1e-6);  loss = sum(m*per_obj)/(sum(m)+1e-6)

Sharding: data-parallel over batch B=8 -> one batch element per NeuronCore.
Each core rasterizes its own 128 objects (object-per-partition layout) and
reduces to (sum m*per_obj, sum m); host adds the 8 partial pairs.

Device algorithm (object k on SBUF partition k, coords un-offset by 32):
  - indirect-DMA gather of pred rows from output[b] transposed to [HW, C]
  - disk AREA (full grid, exact): per (k,y) the 15 disks are x-intervals
    [ceil(cx-h)=floor+1 a.s., floor(cx+h)], h=sqrt(relu(r^2-(y-cy)^2))
    (Act Sqrt); pack (s,e) as s*129+e in int16 (exact), Batcher-sort the
    15 starts + sentinel (10 leveled stages of strided tt min/max, int16
    2x mode), prefix-max the ends, area = sum_j relu(min(Rp_j, s_{j+1})
    - s_j) with a fused accumulator
  - disk BITS only where the IoU intersection needs them: a min-plus
    raster (sqx+sqy broadcast adds + pair-tree, fp16 2x) over the
    polygon-overlap quarter region rows/cols 32:96, sign test on Act
  - polygon: xint'/straddle per (y,v) in fp32; bits = (x < xint') via
    fp16 tensor_tensor; parity via in-place logical_xor pair tree
  - IoU epilogue + masked reduction via PE ones-matmul over partitions
"""

import sys

if "/opt/trn_rl_repo" not in sys.path:
    sys.path.insert(0, "/opt/trn_rl_repo")

import numpy as np

B, C, H, W = 8, 33, 128, 128
K = 128
V = 16          # polygon vertices
D = 15          # disk centers
YC = 16         # disk y-chunk rows
NCH = H // YC   # 16 chunks
DS = 1.0 / 16.0  # disk coordinate scale

_CACHE = {}


def _build_nc():
    import concourse.bacc as bacc
    import concourse.mybir as mybir
    import concourse.tile as tile
    import concourse.bass as bass

    F32 = mybir.dt.float32
    F16 = mybir.dt.float16
    I32 = mybir.dt.int32
    Alu = mybir.AluOpType
    Act = mybir.ActivationFunctionType
    AX = mybir.AxisListType

    nc = bacc.Bacc("TRN2", target_bir_lowering=False, debug=False)

    # ---- DRAM I/O (per core) ----
    featT_d = nc.dram_tensor("featT", [H * W, C], F32, kind="ExternalInput")
    ind_d = nc.dram_tensor("ind", [K], I32, kind="ExternalInput")
    tgt_d = nc.dram_tensor("target", [K, C], F32, kind="ExternalInput")
    mask_d = nc.dram_tensor("mask", [K], I32, kind="ExternalInput")
    out_d = nc.dram_tensor("out", [2], F32, kind="ExternalOutput")

    # ---- SBUF ----
    pred = nc.alloc_sbuf_tensor("pred", [K, C], F32)
    tgt = nc.alloc_sbuf_tensor("tgt", [K, C], F32)
    indc = nc.alloc_sbuf_tensor("indc", [K, 1], I32)
    maski = nc.alloc_sbuf_tensor("maski", [K, 1], I32)
    maskf = nc.alloc_sbuf_tensor("maskf", [K, 1], F32)

    pxi = nc.alloc_sbuf_tensor("pxi", [128, W], I32)
    pxd = nc.alloc_sbuf_tensor("pxd", [128, W], F32)     # x'' = x-32 in [-32,96)

    negc = nc.alloc_sbuf_tensor("negc", [K, 2 * V], F32)  # [-cx_d/16 | -cy_d/16]
    sqx = nc.alloc_sbuf_tensor("sqx", [K, W, D], F16)     # (x,d) d-innermost
    sqy = nc.alloc_sbuf_tensor("sqy", [K, H, D], F16)     # (y,d)
    rsc = nc.alloc_sbuf_tensor("rsc", [K, 4], F32)
    ri = nc.alloc_sbuf_tensor("ri", [K, 1], I32)
    r2c = nc.alloc_sbuf_tensor("r2c", [K, 1], F32)

    slab = nc.alloc_sbuf_tensor("slab", [K, YC, 64, D], F16)
    accq = nc.alloc_sbuf_tensor("accq", [K, YC, 64], F16)
    dk4 = nc.alloc_sbuf_tensor("dk4", [K, 4, YC, 64], F16)
    iscr = nc.alloc_sbuf_tensor("iscr", [K, YC, 64], F16)
    # interval-union area machinery (fp32/int16, [K, H, 15|16] tiles)
    negcu = nc.alloc_sbuf_tensor("negcu", [K, V], F32)
    r2u = nc.alloc_sbuf_tensor("r2u", [K, 1], F32)
    sqyu = nc.alloc_sbuf_tensor("sqyu", [K, H, D], F32)
    hh = nc.alloc_sbuf_tensor("hh", [K, H, D], F32)
    ivA = nc.alloc_sbuf_tensor("ivA", [K, H, 16], F32)
    ivB = nc.alloc_sbuf_tensor("ivB", [K, H, 16], F32)
    ivC = nc.alloc_sbuf_tensor("ivC", [K, H, 16], F32)
    ivI = nc.alloc_sbuf_tensor("ivI", [K, H, 16], I32)
    ivD = nc.alloc_sbuf_tensor("ivD", [K, H, 16], F32)
    I16 = mybir.dt.int16
    pks = nc.alloc_sbuf_tensor("pks", [K, H, 16], I16)
    mtmp = nc.alloc_sbuf_tensor("mtmp", [K, H, 8], I16)

    # polygon
    x2b = nc.alloc_sbuf_tensor("x2b", [K, V], F32)
    y2b = nc.alloc_sbuf_tensor("y2b", [K, V], F32)
    pv1 = nc.alloc_sbuf_tensor("pv1", [K, V], F32)
    pv2 = nc.alloc_sbuf_tensor("pv2", [K, V], F32)
    pv3 = nc.alloc_sbuf_tensor("pv3", [K, V], F32)
    sv = nc.alloc_sbuf_tensor("sv", [K, 64, V], F32)      # (y,v) v-innermost
    svb = nc.alloc_sbuf_tensor("svb", [K, 64, V], F32)
    xint = nc.alloc_sbuf_tensor("xint", [K, 64, V], F32)
    xint16 = nc.alloc_sbuf_tensor("xint16", [K, 64, V], F16)
    pxv16 = nc.alloc_sbuf_tensor("pxv16", [K, 64, V], F16)
    bits = nc.alloc_sbuf_tensor("bits", [K, 16, 64, V], F16)
    gt01 = nc.alloc_sbuf_tensor("gt01", [K, 64, 64], F16)
    gscr = nc.alloc_sbuf_tensor("gscr", [K, 64, 64], F16)

    # reduction buffers
    icols = nc.alloc_sbuf_tensor("icols", [K, max(64 // YC, 2)], F32)
    stats = nc.alloc_sbuf_tensor("stats", [K, 8], F32)
    onesv = nc.alloc_sbuf_tensor("onesv", [K, 1], F32)
    colq = nc.alloc_sbuf_tensor("colq", [K, 2], F32)
    outsb = nc.alloc_sbuf_tensor("outsb", [1, 2], F32)
    psum = nc.alloc_psum_tensor("psum", [1, 2], F32)

    with tile.TileContext(nc) as tc:
        vec = nc.vector
        act = nc.scalar

        def ts(out, in0, s1, op0, s2=None, op1=None, accum=None):
            kw = {}
            if accum is not None:
                kw["accum_out"] = accum
            if op1 is not None:
                return vec.tensor_scalar(out=out, in0=in0, scalar1=s1, scalar2=s2,
                                         op0=op0, op1=op1, **kw)
            return vec.tensor_scalar(out=out, in0=in0, scalar1=s1, scalar2=None,
                                     op0=op0, **kw)

        def tt(out, in0, in1, op):
            return vec.tensor_tensor(out=out, in0=in0, in1=in1, op=op)

        # ---- P0: input DMAs + gather ----
        nc.sync.dma_start(indc.ap(), ind_d.ap().unsqueeze(1))
        nc.sync.dma_start(tgt.ap(), tgt_d.ap())
        nc.sync.dma_start(maski.ap(), mask_d.ap().unsqueeze(1))
        nc.gpsimd.indirect_dma_start(
            out=pred.ap(), out_offset=None, in_=featT_d.ap(),
            in_offset=bass.IndirectOffsetOnAxis(ap=indc.ap(), axis=0))

        # ---- P1: iotas ----
        nc.gpsimd.iota(pxi.ap(), pattern=[[1, W]], base=0, channel_multiplier=0)
        ts(pxd.ap(), pxi.ap(), 32.0, Alu.subtract)          # also int->f32
        ts(maskf.ap(), maski.ap(), 0.0, Alu.add)

        # ---- P3: per-disk squares (scaled by 1/16) ----
        ts(negc.ap()[:, 0:D], pred.ap()[:, 0:2 * D:2], -DS, Alu.mult)
        ts(negc.ap()[:, V:V + D], pred.ap()[:, 1:2 * D:2], -DS, Alu.mult)
        ts(negcu.ap()[:, 0:D], pred.ap()[:, 1:2 * D:2], -1.0, Alu.mult)
        for d in range(D):
            act.activation(out=sqx.ap()[:, :, d], in_=pxd.ap(), func=Act.Square,
                           bias=negc.ap()[:, d:d + 1], scale=DS)
            act.activation(out=sqy.ap()[:, :, d], in_=pxd.ap(), func=Act.Square,
                           bias=negc.ap()[:, V + d:V + d + 1], scale=DS)
            act.activation(out=sqyu.ap()[:, :, d], in_=pxd.ap(), func=Act.Square,
                           bias=negcu.ap()[:, d:d + 1], scale=1.0)

        # ---- P4: polygon precompute (fp32, [K, 64y, V] layout) ----
        x1v = tgt.ap()[:, 0:2 * V:2]     # [K,16]
        y1v = tgt.ap()[:, 1:2 * V:2]
        vec.tensor_copy(out=x2b.ap()[:, 0:V - 1], in_=tgt.ap()[:, 2:2 * V:2])
        vec.tensor_copy(out=x2b.ap()[:, V - 1:V], in_=tgt.ap()[:, 0:1])
        vec.tensor_copy(out=y2b.ap()[:, 0:V - 1], in_=tgt.ap()[:, 3:2 * V:2])
        vec.tensor_copy(out=y2b.ap()[:, V - 1:V], in_=tgt.ap()[:, 1:2])
        d0 = pv1.ap(); eqz = pv2.ap(); sl = pv3.ap()
        tt(d0, y2b.ap(), y1v, Alu.subtract)
        ts(eqz, d0, 0.0, Alu.is_equal)
        tt(d0, d0, eqz, Alu.add)                             # denom
        vec.reciprocal(out=eqz, in_=d0)                      # 1/denom
        tt(sl, x2b.ap(), x1v, Alu.subtract)
        tt(sl, sl, eqz, Alu.mult)                            # slope

        pyp = pxd.ap()[:, 32:96]          # y'' values 0..63
        pyp_b = pyp.unsqueeze(2).to_broadcast([K, 64, V])
        y1b_ = y1v.unsqueeze(1).to_broadcast([K, 64, V])
        y2b_ = y2b.ap().unsqueeze(1).to_broadcast([K, 64, V])
        # straddle = (y1 > y) != (y2 > y)
        tt(sv.ap(), y1b_, pyp_b, Alu.is_gt)
        tt(svb.ap(), y2b_, pyp_b, Alu.is_gt)
        tt(sv.ap(), sv.ap(), svb.ap(), Alu.not_equal)
        # xint = x1 + (y - y1)*slope ; xint' = straddle * xint (in (0,64) when straddle)
        tt(xint.ap(), pyp_b, y1b_, Alu.subtract)
        tt(xint.ap(), xint.ap(), sl.unsqueeze(1).to_broadcast([K, 64, V]), Alu.mult)
        tt(xint.ap(), xint.ap(), x1v.unsqueeze(1).to_broadcast([K, 64, V]), Alu.add)
        tt(xint16.ap(), xint.ap(), sv.ap(), Alu.mult)
        # pxv16[k, x, v] = x'' (0..63)
        ts(pxv16.ap(), pxd.ap()[:, 32:96].unsqueeze(2).to_broadcast([K, 64, V]),
           0.0, Alu.add)

        # ---- P5: polygon bits + xor-tree parity ----
        pxv_b = pxv16.ap().unsqueeze(1).to_broadcast([K, 16, 64, V])
        for sc in range(4):
            xv = xint16.ap()[:, 16 * sc:16 * sc + 16, :].unsqueeze(2) \
                .to_broadcast([K, 16, 64, V])
            tt(bits.ap(), pxv_b, xv, Alu.is_lt)              # x < xint'
            tt(bits.ap()[:, :, :, 0:8], bits.ap()[:, :, :, 0:8],
               bits.ap()[:, :, :, 8:16], Alu.logical_xor)
            tt(bits.ap()[:, :, :, 0:4], bits.ap()[:, :, :, 0:4],
               bits.ap()[:, :, :, 4:8], Alu.logical_xor)
            tt(bits.ap()[:, :, :, 0:2], bits.ap()[:, :, :, 0:2],
               bits.ap()[:, :, :, 2:4], Alu.logical_xor)
            tt(gt01.ap()[:, 16 * sc:16 * sc + 16, :],
               bits.ap()[:, :, :, 0], bits.ap()[:, :, :, 1], Alu.logical_xor)
        # area_gt (bits are exact 0/1 in fp16)
        act.activation(out=gscr.ap(), in_=gt01.ap(), func=Act.Identity,
                       bias=0.0, scale=1.0, accum_out=stats.ap()[:, 2:3])

        # ---- P2: r2c = (ceil(|pred[:,32]|)/16)^2  (cast-based floor) ----
        u = rsc.ap()[:, 0:1]; t = rsc.ap()[:, 1:2]; g = rsc.ap()[:, 2:3]
        ts(t, pred.ap()[:, 32:33], -1.0, Alu.mult)
        tt(u, pred.ap()[:, 32:33], t, Alu.max)              # |p|
        vec.tensor_copy(out=ri.ap(), in_=u)                 # int cast
        vec.tensor_copy(out=t, in_=ri.ap())                 # back to f32
        tt(g, t, u, Alu.is_gt)
        tt(t, t, g, Alu.subtract)                           # floor(u)
        tt(g, u, t, Alu.is_gt)
        tt(t, t, g, Alu.add)                                # ceil(u)
        tt(r2u.ap(), t, t, Alu.mult)                        # r^2 (unscaled)
        ts(t, t, DS, Alu.mult)
        tt(r2c.ap(), t, t, Alu.mult)                        # (r/16)^2

        # ---- P6a: quarter-region raster (poly-overlap rows 32:96, cols 32:96) ----
        sqx_b = sqx.ap()[:, 32:96, :].unsqueeze(1).to_broadcast([K, YC, 64, D])
        for j in range(64 // YC):
            sqy_b = sqy.ap()[:, 32 + YC * j:32 + YC * (j + 1), :].unsqueeze(2) \
                .to_broadcast([K, YC, 64, D])
            tt(slab.ap(), sqx_b, sqy_b, Alu.add)
            tt(slab.ap()[:, :, :, 0:7], slab.ap()[:, :, :, 0:7],
               slab.ap()[:, :, :, 8:15], Alu.min)
            tt(slab.ap()[:, :, :, 0:4], slab.ap()[:, :, :, 0:4],
               slab.ap()[:, :, :, 4:8], Alu.min)
            tt(slab.ap()[:, :, :, 0:2], slab.ap()[:, :, :, 0:2],
               slab.ap()[:, :, :, 2:4], Alu.min)
            tt(accq.ap(), slab.ap()[:, :, :, 0], slab.ap()[:, :, :, 1], Alu.min)
            act.activation(out=dk4.ap()[:, j, :, :], in_=accq.ap(), func=Act.Sign,
                           bias=r2c.ap(), scale=-1.0)
        for j in range(64 // YC):
            tt(iscr.ap(), dk4.ap()[:, j, :, :],
               gt01.ap()[:, YC * j:YC * (j + 1), :], Alu.mult)
            vec.tensor_scalar(out=iscr.ap(), in0=iscr.ap(), scalar1=0.0,
                              scalar2=None, op0=Alu.add, op1=Alu.add,
                              accum_out=icols.ap()[:, j:j + 1])

        # ---- P6b: exact full-grid disk area via per-row interval union ----
        # per (k, y, d): global-x interval [ceil(cxg-h), floor(cxg+h)],
        # h = sqrt(relu(r^2-(y-cy)^2)); pack (s,e) as s*129+e in int16,
        # Batcher-sort the 15 starts (+sentinel), prefix-max ends, sum runs.
        ts(x2b.ap()[:, 0:D], pred.ap()[:, 0:2 * D:2], 32.0, Alu.add)  # cxg; x2b is free here
        cxg_b = x2b.ap()[:, 0:D].unsqueeze(1).to_broadcast([K, H, D])
        ts(hh.ap(), sqyu.ap(), -1.0, Alu.mult, r2u.ap(), Alu.add)
        ts(hh.ap(), hh.ap(), 0.0, Alu.max)
        act.activation(out=hh.ap(), in_=hh.ap(), func=Act.Sqrt)
        loA = ivA.ap()[:, :, 0:D]; fB = ivB.ap()[:, :, 0:D]
        cC = ivC.ap()[:, :, 0:D]; gD = ivD.ap()[:, :, 0:D]
        iI = ivI.ap()[:, :, 0:D]
        tt(loA, cxg_b, hh.ap(), Alu.subtract)
        tt(fB, cxg_b, hh.ap(), Alu.add)
        # s = ceil(lo) = floor(lo)+1 a.s. (lo continuous), clipped to [0,128]
        vec.tensor_copy(out=iI, in_=loA)
        vec.tensor_copy(out=cC, in_=iI)
        tt(gD, cC, loA, Alu.is_gt)
        tt(cC, cC, gD, Alu.subtract)          # floor(lo)
        ts(loA, cC, 1.0, Alu.add, 0.0, Alu.max)
        ts(loA, loA, 128.0, Alu.min)
        # e = floor(hi)+1, clipped to [0,128], >= s
        vec.tensor_copy(out=iI, in_=fB)
        vec.tensor_copy(out=cC, in_=iI)
        tt(gD, cC, fB, Alu.is_gt)
        tt(cC, cC, gD, Alu.subtract)          # floor(hi)
        ts(fB, cC, 1.0, Alu.add, 0.0, Alu.max)
        ts(fB, fB, 128.0, Alu.min)
        tt(fB, fB, loA, Alu.max)
        # pack and sort
        vec.scalar_tensor_tensor(out=cC, in0=loA, scalar=129.0, in1=fB,
                                 op0=Alu.mult, op1=Alu.add)
        vec.tensor_copy(out=pks.ap()[:, :, 0:D], in_=cC)
        vec.memset(pks.ap()[:, :, D:16], 16640)
        tt(mtmp.ap()[:, :, 0:8], pks.ap()[:, :, 0:15:2], pks.ap()[:, :, 1:16:2], Alu.min)
        tt(pks.ap()[:, :, 1:16:2], pks.ap()[:, :, 0:15:2], pks.ap()[:, :, 1:16:2], Alu.max)
        vec.tensor_copy(out=pks.ap()[:, :, 0:15:2], in_=mtmp.ap()[:, :, 0:8])
        tt(mtmp.ap()[:, :, 0:2], pks.ap()[:, :, 0:2], pks.ap()[:, :, 2:4], Alu.min)
        tt(pks.ap()[:, :, 2:4], pks.ap()[:, :, 0:2], pks.ap()[:, :, 2:4], Alu.max)
        vec.tensor_copy(out=pks.ap()[:, :, 0:2], in_=mtmp.ap()[:, :, 0:2])
        tt(mtmp.ap()[:, :, 0:2], pks.ap()[:, :, 4:6], pks.ap()[:, :, 6:8], Alu.min)
        tt(pks.ap()[:, :, 6:8], pks.ap()[:, :, 4:6], pks.ap()[:, :, 6:8], Alu.max)
        vec.tensor_copy(out=pks.ap()[:, :, 4:6], in_=mtmp.ap()[:, :, 0:2])
        tt(mtmp.ap()[:, :, 0:2], pks.ap()[:, :, 8:10], pks.ap()[:, :, 10:12], Alu.min)
        tt(pks.ap()[:, :, 10:12], pks.ap()[:, :, 8:10], pks.ap()[:, :, 10:12], Alu.max)
        vec.tensor_copy(out=pks.ap()[:, :, 8:10], in_=mtmp.ap()[:, :, 0:2])
        tt(mtmp.ap()[:, :, 0:2], pks.ap()[:, :, 12:14], pks.ap()[:, :, 14:16], Alu.min)
        tt(pks.ap()[:, :, 14:16], pks.ap()[:, :, 12:14], pks.ap()[:, :, 14:16], Alu.max)
        vec.tensor_copy(out=pks.ap()[:, :, 12:14], in_=mtmp.ap()[:, :, 0:2])
        tt(mtmp.ap()[:, :, 0:4], pks.ap()[:, :, 1:14:4], pks.ap()[:, :, 2:15:4], Alu.min)
        tt(pks.ap()[:, :, 2:15:4], pks.ap()[:, :, 1:14:4], pks.ap()[:, :, 2:15:4], Alu.max)
        vec.tensor_copy(out=pks.ap()[:, :, 1:14:4], in_=mtmp.ap()[:, :, 0:4])
        tt(mtmp.ap()[:, :, 0:2], pks.ap()[:, :, 0:4:3], pks.ap()[:, :, 4:8:3], Alu.min)
        tt(pks.ap()[:, :, 4:8:3], pks.ap()[:, :, 0:4:3], pks.ap()[:, :, 4:8:3], Alu.max)
        vec.tensor_copy(out=pks.ap()[:, :, 0:4:3], in_=mtmp.ap()[:, :, 0:2])
        tt(mtmp.ap()[:, :, 0:2], pks.ap()[:, :, 8:12:3], pks.ap()[:, :, 12:16:3], Alu.min)
        tt(pks.ap()[:, :, 12:16:3], pks.ap()[:, :, 8:12:3], pks.ap()[:, :, 12:16:3], Alu.max)
        vec.tensor_copy(out=pks.ap()[:, :, 8:12:3], in_=mtmp.ap()[:, :, 0:2])
        tt(mtmp.ap()[:, :, 0:2], pks.ap()[:, :, 1:3], pks.ap()[:, :, 5:7], Alu.min)
        tt(pks.ap()[:, :, 5:7], pks.ap()[:, :, 1:3], pks.ap()[:, :, 5:7], Alu.max)
        vec.tensor_copy(out=pks.ap()[:, :, 1:3], in_=mtmp.ap()[:, :, 0:2])
        tt(mtmp.ap()[:, :, 0:2], pks.ap()[:, :, 9:11], pks.ap()[:, :, 13:15], Alu.min)
        tt(pks.ap()[:, :, 13:15], pks.ap()[:, :, 9:11], pks.ap()[:, :, 13:15], Alu.max)
        vec.tensor_copy(out=pks.ap()[:, :, 9:11], in_=mtmp.ap()[:, :, 0:2])
        tt(mtmp.ap()[:, :, 0:2], pks.ap()[:, :, 0:8:7], pks.ap()[:, :, 8:16:7], Alu.min)
        tt(pks.ap()[:, :, 8:16:7], pks.ap()[:, :, 0:8:7], pks.ap()[:, :, 8:16:7], Alu.max)
        vec.tensor_copy(out=pks.ap()[:, :, 0:8:7], in_=mtmp.ap()[:, :, 0:2])
        tt(mtmp.ap()[:, :, 0:2], pks.ap()[:, :, 2:4], pks.ap()[:, :, 4:6], Alu.min)
        tt(pks.ap()[:, :, 4:6], pks.ap()[:, :, 2:4], pks.ap()[:, :, 4:6], Alu.max)
        vec.tensor_copy(out=pks.ap()[:, :, 2:4], in_=mtmp.ap()[:, :, 0:2])
        tt(mtmp.ap()[:, :, 0:2], pks.ap()[:, :, 10:12], pks.ap()[:, :, 12:14], Alu.min)
        tt(pks.ap()[:, :, 12:14], pks.ap()[:, :, 10:12], pks.ap()[:, :, 12:14], Alu.max)
        vec.tensor_copy(out=pks.ap()[:, :, 10:12], in_=mtmp.ap()[:, :, 0:2])
        tt(mtmp.ap()[:, :, 0:3], pks.ap()[:, :, 1:6:2], pks.ap()[:, :, 2:7:2], Alu.min)
        tt(pks.ap()[:, :, 2:7:2], pks.ap()[:, :, 1:6:2], pks.ap()[:, :, 2:7:2], Alu.max)
        vec.tensor_copy(out=pks.ap()[:, :, 1:6:2], in_=mtmp.ap()[:, :, 0:3])
        tt(mtmp.ap()[:, :, 0:3], pks.ap()[:, :, 9:14:2], pks.ap()[:, :, 10:15:2], Alu.min)
        tt(pks.ap()[:, :, 10:15:2], pks.ap()[:, :, 9:14:2], pks.ap()[:, :, 10:15:2], Alu.max)
        vec.tensor_copy(out=pks.ap()[:, :, 9:14:2], in_=mtmp.ap()[:, :, 0:3])
        tt(mtmp.ap()[:, :, 0:6], pks.ap()[:, :, 1:7], pks.ap()[:, :, 9:15], Alu.min)
        tt(pks.ap()[:, :, 9:15], pks.ap()[:, :, 1:7], pks.ap()[:, :, 9:15], Alu.max)
        vec.tensor_copy(out=pks.ap()[:, :, 1:7], in_=mtmp.ap()[:, :, 0:6])
        tt(mtmp.ap()[:, :, 0:4], pks.ap()[:, :, 4:8], pks.ap()[:, :, 8:12], Alu.min)
        tt(pks.ap()[:, :, 8:12], pks.ap()[:, :, 4:8], pks.ap()[:, :, 8:12], Alu.max)
        vec.tensor_copy(out=pks.ap()[:, :, 4:8], in_=mtmp.ap()[:, :, 0:4])
        tt(mtmp.ap()[:, :, 0:2], pks.ap()[:, :, 2:4], pks.ap()[:, :, 4:6], Alu.min)
        tt(pks.ap()[:, :, 4:6], pks.ap()[:, :, 2:4], pks.ap()[:, :, 4:6], Alu.max)
        vec.tensor_copy(out=pks.ap()[:, :, 2:4], in_=mtmp.ap()[:, :, 0:2])
        tt(mtmp.ap()[:, :, 0:2], pks.ap()[:, :, 6:8], pks.ap()[:, :, 8:10], Alu.min)
        tt(pks.ap()[:, :, 8:10], pks.ap()[:, :, 6:8], pks.ap()[:, :, 8:10], Alu.max)
        vec.tensor_copy(out=pks.ap()[:, :, 6:8], in_=mtmp.ap()[:, :, 0:2])
        tt(mtmp.ap()[:, :, 0:2], pks.ap()[:, :, 10:12], pks.ap()[:, :, 12:14], Alu.min)
        tt(pks.ap()[:, :, 12:14], pks.ap()[:, :, 10:12], pks.ap()[:, :, 12:14], Alu.max)
        vec.tensor_copy(out=pks.ap()[:, :, 10:12], in_=mtmp.ap()[:, :, 0:2])
        tt(mtmp.ap()[:, :, 0:7], pks.ap()[:, :, 1:14:2], pks.ap()[:, :, 2:15:2], Alu.min)
        tt(pks.ap()[:, :, 2:15:2], pks.ap()[:, :, 1:14:2], pks.ap()[:, :, 2:15:2], Alu.max)
        vec.tensor_copy(out=pks.ap()[:, :, 1:14:2], in_=mtmp.ap()[:, :, 0:7])
        # unpack: s = floor((pk+0.5)/129), e = pk - 129*s
        vec.tensor_copy(out=ivC.ap(), in_=pks.ap())
        ts(ivA.ap(), ivC.ap(), 1.0 / 129.0, Alu.mult, 0.5 / 129.0, Alu.add)
        vec.tensor_copy(out=ivI.ap(), in_=ivA.ap())
        vec.tensor_copy(out=ivB.ap(), in_=ivI.ap())
        tt(ivD.ap(), ivB.ap(), ivA.ap(), Alu.is_gt)
        tt(ivB.ap(), ivB.ap(), ivD.ap(), Alu.subtract)   # s (sorted)
        vec.scalar_tensor_tensor(out=ivD.ap(), in0=ivB.ap(), scalar=-129.0,
                                 in1=ivC.ap(), op0=Alu.mult, op1=Alu.add)  # e
        # prefix-max of e along slots (ping-pong ivD <-> ivC)
        tt(ivC.ap()[:, :, 1:16], ivD.ap()[:, :, 1:16], ivD.ap()[:, :, 0:15], Alu.max)
        vec.tensor_copy(out=ivC.ap()[:, :, 0:1], in_=ivD.ap()[:, :, 0:1])
        tt(ivD.ap()[:, :, 2:16], ivC.ap()[:, :, 2:16], ivC.ap()[:, :, 0:14], Alu.max)
        vec.tensor_copy(out=ivD.ap()[:, :, 0:2], in_=ivC.ap()[:, :, 0:2])
        tt(ivC.ap()[:, :, 4:16], ivD.ap()[:, :, 4:16], ivD.ap()[:, :, 0:12], Alu.max)
        vec.tensor_copy(out=ivC.ap()[:, :, 0:4], in_=ivD.ap()[:, :, 0:4])
        tt(ivD.ap()[:, :, 8:16], ivC.ap()[:, :, 8:16], ivC.ap()[:, :, 0:8], Alu.max)
        vec.tensor_copy(out=ivD.ap()[:, :, 0:8], in_=ivC.ap()[:, :, 0:8])
        # covered = sum_j relu(min(Rp_j, s_{j+1}) - s_j)
        tt(ivC.ap()[:, :, 0:D], ivD.ap()[:, :, 0:D], ivB.ap()[:, :, 1:16], Alu.min)
        tt(ivC.ap()[:, :, 0:D], ivC.ap()[:, :, 0:D], ivB.ap()[:, :, 0:D],
           Alu.subtract)
        vec.tensor_scalar(out=ivC.ap()[:, :, 0:D], in0=ivC.ap()[:, :, 0:D],
                          scalar1=0.0, scalar2=None, op0=Alu.max, op1=Alu.add,
                          accum_out=stats.ap()[:, 0:1])   # area_dk

        # ---- P7: epilogue ----
        adk = stats.ap()[:, 0:1]; itr = stats.ap()[:, 1:2]; agt = stats.ap()[:, 2:3]
        uni = stats.ap()[:, 3:4]; den = stats.ap()[:, 4:5]; pob = stats.ap()[:, 5:6]
        vec.tensor_reduce(out=itr, in_=icols.ap(), axis=AX.X, op=Alu.add)
        tt(itr, itr, agt, Alu.add)
        ts(itr, itr, 0.5, Alu.mult)
        tt(uni, adk, agt, Alu.add)
        tt(uni, uni, itr, Alu.subtract)
        ts(den, uni, 1e-6, Alu.add)
        vec.reciprocal(out=den, in_=den)
        tt(pob, itr, den, Alu.mult)
        ts(pob, pob, -1.0, Alu.mult, 1.0, Alu.add)        # 1 - inter/union
        tt(colq.ap()[:, 0:1], pob, maskf.ap(), Alu.mult)
        vec.tensor_copy(out=colq.ap()[:, 1:2], in_=maskf.ap())
        vec.memset(onesv.ap(), 1.0)
        nc.tensor.matmul(out=psum.ap(), lhsT=onesv.ap(), rhs=colq.ap(),
                         start=True, stop=True)
        vec.tensor_copy(out=outsb.ap(), in_=psum.ap())
        nc.sync.dma_start(out_d.ap().unsqueeze(0), outsb.ap())

    nc.compile()
    return nc


def _get_nc():
    if "nc" not in _CACHE:
        _CACHE["nc"] = _build_nc()
    return _CACHE["nc"]


def kernel(output, mask, ind, target, freq_mask=None):
    nc = _get_nc()
    from concourse.bass_utils import run_bass_kernel_spmd

    output = np.asarray(output, dtype=np.float32)
    target = np.asarray(target, dtype=np.float32)
    in_maps = []
    for b in range(B):
        in_maps.append({
            "featT": np.ascontiguousarray(output[b].reshape(C, H * W).T),
            "ind": np.asarray(ind[b], dtype=np.int32),
            "target": np.ascontiguousarray(target[b]),
            "mask": np.asarray(mask[b], dtype=np.int32),
        })
    res = run_bass_kernel_spmd(nc, in_maps, core_ids=list(range(B)))
    parts = np.stack([np.asarray(r["out"], dtype=np.float64) for r in res.results])
    loss = parts[:, 0].sum() / (parts[:, 1].sum() + 1e-6)
    return np.float32(loss), np.float32(0.0)



# revision 10
# speedup vs baseline: 2.2321x; 2.2321x over previous
"""DiskLoss Trainium2 kernel (interval-union formulation).

Computes the reference loss:
  pred = gather(output, ind)            # [K,33] per batch
  gt_m = even-odd rasterization of the 16-gon from target   (per object)
  dk_m = union of 15 disks (radius ceil(|pred[:,32]|)) from pred
  per_obj = 1 - inter/(union+1e-6);  loss = sum(m*per_obj)/(sum(m)+1e-6)

Sharding: data-parallel over batch B=8 -> one batch element per NeuronCore.
Each core reduces its 128 objects (object-per-partition layout) to
(sum m*per_obj, sum m); host adds the 8 partial pairs.

Device algorithm — both masks are per-row interval unions, no pixel raster:
  - disks: per (k,y,d) the row span is [s,e) with h=sqrt(relu(r^2-(y-cy)^2)),
    s=max(0,floor(cx-h)+1), e=min(128,floor(cx+h)+1); floor/clip via ACT
    round-to-nearest int conversion (+-0.5 bias) with uint8 saturation;
    pack p=s*129+(e-128) int16, Batcher-sort the 16 slots per row, unpack
    via ACT, prefix-max ends -> disjoint runs [s'_j, u'_j) (values shifted
    by -128); disk area = sum relu(u'-s')
  - polygon: crossings xint per (y in 32:96, v) in fp32; straddle via
    (y-y1)(y-y2)<0; c-128 = round(xint-95.5) via ACT, zeroed (sentinel)
    unless straddle, packed *130 and sorted in the same Batcher pass
    (rows 128:192); even-odd pairs (a_i,b_i) of the sorted crossings are
    the disjoint poly intervals; poly area = sum(b-a)
  - intersection = sum_{i,j} relu(min(b_i,u'_j) - max(a_i,s'_j)) over the
    poly-overlap rows 32:96 (8 poly pairs x 15 disk runs, int16)
  - DVE does min/max/sort; Pool (gpsimd) does arithmetic (add/sub/mult
    only on this ISA); ACT does all float->int rounding; PE does the final
    masked reduction via ones-matmul over partitions
"""

import sys

if "/opt/trn_rl_repo" not in sys.path:
    sys.path.insert(0, "/opt/trn_rl_repo")

import numpy as np

B, C, H, W = 8, 33, 128, 128
K = 128
V = 16          # polygon vertices
D = 15          # disk centers

_CACHE = {}


def _build_nc():
    import concourse.bacc as bacc
    import concourse.mybir as mybir
    import concourse.tile as tile
    import concourse.bass as bass

    F32 = mybir.dt.float32
    F16 = mybir.dt.float16
    I32 = mybir.dt.int32
    I16 = mybir.dt.int16
    U8 = mybir.dt.uint8
    Alu = mybir.AluOpType
    Act = mybir.ActivationFunctionType

    nc = bacc.Bacc("TRN2", target_bir_lowering=False, debug=False)

    # ---- DRAM I/O (per core) ----
    featT_d = nc.dram_tensor("featT", [H * W, C], F32, kind="ExternalInput")
    ind_d = nc.dram_tensor("ind", [K], I32, kind="ExternalInput")
    tgt_d = nc.dram_tensor("target", [K, C], F32, kind="ExternalInput")
    mask_d = nc.dram_tensor("mask", [K], I32, kind="ExternalInput")
    out_d = nc.dram_tensor("out", [2], F32, kind="ExternalOutput")

    # ---- SBUF ----
    pred = nc.alloc_sbuf_tensor("pred", [K, C], F32)
    tgt = nc.alloc_sbuf_tensor("tgt", [K, C], F32)
    indc = nc.alloc_sbuf_tensor("indc", [K, 1], I32)
    maski = nc.alloc_sbuf_tensor("maski", [K, 1], I32)
    maskf = nc.alloc_sbuf_tensor("maskf", [K, 1], F32)

    pxi = nc.alloc_sbuf_tensor("pxi", [K, W], I32)
    pyg = nc.alloc_sbuf_tensor("pyg", [K, W], F32)      # y global 0..127
    pysh = nc.alloc_sbuf_tensor("pysh", [K, 64], F32)   # y-32 for rows 32:96

    # disk geometry
    negcu = nc.alloc_sbuf_tensor("negcu", [K, D], F32)  # -(cy+32)
    cxg = nc.alloc_sbuf_tensor("cxg", [K, D], F32)      # cx+32
    rsc = nc.alloc_sbuf_tensor("rsc", [K, 4], F32)
    ri = nc.alloc_sbuf_tensor("ri", [K, 1], I32)
    r2u = nc.alloc_sbuf_tensor("r2u", [K, 1], F32)
    sqyu = nc.alloc_sbuf_tensor("sqyu", [K, H, D], F32)
    hsq = nc.alloc_sbuf_tensor("hsq", [K, H, D], F32)
    hh = nc.alloc_sbuf_tensor("hh", [K, H, D], F32)
    lo = nc.alloc_sbuf_tensor("lo", [K, H, D], F32)
    hi = nc.alloc_sbuf_tensor("hi", [K, H, D], F32)
    s8 = nc.alloc_sbuf_tensor("s8", [K, H, D], U8)
    e8 = nc.alloc_sbuf_tensor("e8", [K, H, D], U8)

    # polygon geometry
    x2b = nc.alloc_sbuf_tensor("x2b", [K, V], F32)
    y2b = nc.alloc_sbuf_tensor("y2b", [K, V], F32)
    pv1 = nc.alloc_sbuf_tensor("pv1", [K, V], F32)
    pv2 = nc.alloc_sbuf_tensor("pv2", [K, V], F32)
    pv3 = nc.alloc_sbuf_tensor("pv3", [K, V], F32)
    xa = nc.alloc_sbuf_tensor("xa", [K, 64, V], F32)
    ya = nc.alloc_sbuf_tensor("ya", [K, 64, V], F32)
    xb = nc.alloc_sbuf_tensor("xb", [K, 64, V], F32)
    nei = nc.alloc_sbuf_tensor("nei", [K, 64, V], I16)
    cpre = nc.alloc_sbuf_tensor("cpre", [K, 64, V], I16)

    # sort + runs (all int16)
    pks = nc.alloc_sbuf_tensor("pks", [K, 192, 16], I16)
    mtA = nc.alloc_sbuf_tensor("mtA", [K, 192, 8], I16)
    sshift = nc.alloc_sbuf_tensor("sshift", [K, H, 16], I16)   # s-128 (disk)
    suc = nc.alloc_sbuf_tensor("suc", [K, 192, 16], I16)       # s | c-128
    etld = nc.alloc_sbuf_tensor("etld", [K, H, 16], I16)       # e-128 sorted
    ebuf = nc.alloc_sbuf_tensor("ebuf", [K, H, 16], I16)
    uu = nc.alloc_sbuf_tensor("uu", [K, H, D], I16)
    dd = nc.alloc_sbuf_tensor("dd", [K, H, D], I16)

    # intersection
    mx = nc.alloc_sbuf_tensor("mx", [K, 64, 8, D], I16)
    mn = nc.alloc_sbuf_tensor("mn", [K, 64, 8, D], I16)
    df = nc.alloc_sbuf_tensor("df", [K, 64, 8, D], I16)

    # act bias constants
    bm95 = nc.alloc_sbuf_tensor("bm95", [K, 1], F32)     # -95.5 (poly c shift)
    bp05 = nc.alloc_sbuf_tensor("bp05", [K, 1], F32)     # +0.5
    b1275 = nc.alloc_sbuf_tensor("b1275", [K, 1], F32)   # +127.5
    bunA = nc.alloc_sbuf_tensor("bunA", [K, 1], F32)     # unpack shifted
    bunB = nc.alloc_sbuf_tensor("bunB", [K, 1], F32)     # unpack unshifted

    # reduction
    stats = nc.alloc_sbuf_tensor("stats", [K, 8], F32)
    onesv = nc.alloc_sbuf_tensor("onesv", [K, 1], F32)
    colq = nc.alloc_sbuf_tensor("colq", [K, 2], F32)
    outsb = nc.alloc_sbuf_tensor("outsb", [1, 2], F32)
    psum = nc.alloc_psum_tensor("psum", [1, 2], F32)

    with tile.TileContext(nc) as tc:
        vec = nc.vector
        gps = nc.gpsimd
        act = nc.scalar

        def ts(eng, out, in0, s1, op0, s2=None, op1=None, accum=None):
            kw = {}
            if accum is not None:
                kw["accum_out"] = accum
            if op1 is not None:
                return eng.tensor_scalar(out=out, in0=in0, scalar1=s1, scalar2=s2,
                                         op0=op0, op1=op1, **kw)
            return eng.tensor_scalar(out=out, in0=in0, scalar1=s1, scalar2=None,
                                     op0=op0, **kw)

        def tt(eng, out, in0, in1, op):
            return eng.tensor_tensor(out=out, in0=in0, in1=in1, op=op)

        # ---- P0: input DMAs + gather + iotas + consts ----
        nc.sync.dma_start(indc.ap(), ind_d.ap().unsqueeze(1))
        nc.sync.dma_start(tgt.ap(), tgt_d.ap())
        nc.sync.dma_start(maski.ap(), mask_d.ap().unsqueeze(1))
        nc.gpsimd.indirect_dma_start(
            out=pred.ap(), out_offset=None, in_=featT_d.ap(),
            in_offset=bass.IndirectOffsetOnAxis(ap=indc.ap(), axis=0))

        nc.gpsimd.iota(pxi.ap(), pattern=[[1, W]], base=0, channel_multiplier=0)
        ts(vec, pyg.ap(), pxi.ap(), 0.0, Alu.add)            # int->f32, 0..127
        ts(vec, pysh.ap(), pxi.ap()[:, 32:96], -32.0, Alu.add)
        ts(vec, maskf.ap(), maski.ap(), 0.0, Alu.add)
        vec.memset(bm95.ap(), -95.5)
        vec.memset(bp05.ap(), 0.5)
        vec.memset(b1275.ap(), 127.5)
        vec.memset(bunA.ap(), 128.0 / 129.0 - 0.5 - 128.0)
        vec.memset(bunB.ap(), 128.0 / 129.0 - 0.5)
        vec.memset(onesv.ap(), 1.0)

        # ---- P1: polygon precompute (raw coords; rows are y-32 in 0..63) ----
        x1v = tgt.ap()[:, 0:2 * V:2]
        y1v = tgt.ap()[:, 1:2 * V:2]
        gps.tensor_copy(out=x2b.ap()[:, 0:V - 1], in_=tgt.ap()[:, 2:2 * V:2])
        gps.tensor_copy(out=x2b.ap()[:, V - 1:V], in_=tgt.ap()[:, 0:1])
        gps.tensor_copy(out=y2b.ap()[:, 0:V - 1], in_=tgt.ap()[:, 3:2 * V:2])
        gps.tensor_copy(out=y2b.ap()[:, V - 1:V], in_=tgt.ap()[:, 1:2])
        d0 = pv1.ap(); eqz = pv2.ap(); sl = pv3.ap()
        tt(vec, d0, y2b.ap(), y1v, Alu.subtract)
        ts(vec, eqz, d0, 0.0, Alu.is_equal)
        tt(vec, d0, d0, eqz, Alu.add)                        # denom
        vec.reciprocal(out=eqz, in_=d0)                      # 1/denom
        tt(vec, sl, x2b.ap(), x1v, Alu.subtract)
        tt(vec, sl, sl, eqz, Alu.mult)                       # slope

        pyb = pysh.ap().unsqueeze(2).to_broadcast([K, 64, V])
        y1b = y1v.unsqueeze(1).to_broadcast([K, 64, V])
        y2bb = y2b.ap().unsqueeze(1).to_broadcast([K, 64, V])
        # straddle = (y-y1)(y-y2) < 0  (a.s. equal to reference predicate)
        tt(gps, xa.ap(), pyb, y1b, Alu.subtract)             # y-y1 (also xint)
        tt(gps, ya.ap(), pyb, y2bb, Alu.subtract)            # y-y2
        tt(gps, ya.ap(), ya.ap(), xa.ap(), Alu.mult)
        ts(vec, nei.ap(), ya.ap(), 0.0, Alu.is_lt)           # straddle 0/1 i16
        # xint (raw coords) = x1 + (y - y1)*slope
        tt(vec, xb.ap(), xa.ap(), sl.unsqueeze(1).to_broadcast([K, 64, V]), Alu.mult)
        tt(gps, xa.ap(), xb.ap(), x1v.unsqueeze(1).to_broadcast([K, 64, V]), Alu.add)
        # c-128 = round(xint_raw - 95.5); garbage for non-straddle (zeroed next)
        act.activation(out=cpre.ap(), in_=xa.ap(), func=Act.Identity,
                       bias=bm95.ap(), scale=1.0)
        # pack poly rows: p = (c-128)*130 * straddle
        vec.scalar_tensor_tensor(out=pks.ap()[:, 128:192, :], in0=cpre.ap(),
                                 scalar=130.0, in1=nei.ap(),
                                 op0=Alu.mult, op1=Alu.mult)

        # ---- P2: disk geometry ----
        ts(vec, negcu.ap(), pred.ap()[:, 1:2 * D:2], -1.0, Alu.mult, -32.0, Alu.add)
        ts(vec, cxg.ap(), pred.ap()[:, 0:2 * D:2], 32.0, Alu.add)
        u = rsc.ap()[:, 0:1]; t = rsc.ap()[:, 1:2]; g = rsc.ap()[:, 2:3]
        ts(vec, t, pred.ap()[:, 32:33], -1.0, Alu.mult)
        tt(vec, u, pred.ap()[:, 32:33], t, Alu.max)          # |p|
        vec.tensor_copy(out=ri.ap(), in_=u)
        vec.tensor_copy(out=t, in_=ri.ap())
        tt(vec, g, t, u, Alu.is_gt)
        tt(vec, t, t, g, Alu.subtract)                       # floor
        tt(vec, g, u, t, Alu.is_gt)
        tt(vec, t, t, g, Alu.add)                            # ceil = r
        tt(vec, r2u.ap(), t, t, Alu.mult)                    # r^2

        for d in range(D):
            act.activation(out=sqyu.ap()[:, :, d], in_=pyg.ap(), func=Act.Square,
                           bias=negcu.ap()[:, d:d + 1], scale=1.0)
        # hsqn = min(sqyu - r^2, 0);  h = sqrt(-hsqn)
        ts(vec, hsq.ap(), sqyu.ap(), r2u.ap(), Alu.subtract, 0.0, Alu.min)
        act.activation(out=hh.ap(), in_=hsq.ap(), func=Act.Sqrt,
                       bias=0.0, scale=-1.0)
        cxb = cxg.ap().unsqueeze(1).to_broadcast([K, H, D])
        tt(gps, lo.ap(), cxb, hh.ap(), Alu.subtract)
        tt(vec, hi.ap(), cxb, hh.ap(), Alu.add)
        # s = round(lo+0.5) sat to [0,255];  e_rev = round(127.5-hi) sat
        act.activation(out=s8.ap(), in_=lo.ap(), func=Act.Identity,
                       bias=bp05.ap(), scale=1.0)
        act.activation(out=e8.ap(), in_=hi.ap(), func=Act.Identity,
                       bias=b1275.ap(), scale=-1.0)
        # pack disk rows: p = s*129 - e_rev = s*129 + (e-128)
        vec.scalar_tensor_tensor(out=pks.ap()[:, 0:128, 0:D], in0=s8.ap(),
                                 scalar=129.0, in1=e8.ap(),
                                 op0=Alu.mult, op1=Alu.subtract)
        vec.memset(pks.ap()[:, 0:128, D:16], 16512)          # s=128,e=128 sentinel

        # ---- P3: Batcher odd-even mergesort of the 16 slots (all rows) ----
        GROUPS = [
            ((0, 15, 2), (1, 16, 2), 8),
            ((0, 2, 1), (2, 4, 1), 2),
            ((4, 6, 1), (6, 8, 1), 2),
            ((8, 10, 1), (10, 12, 1), 2),
            ((12, 14, 1), (14, 16, 1), 2),
            ((1, 14, 4), (2, 15, 4), 4),
            ((0, 4, 3), (4, 8, 3), 2),
            ((8, 12, 3), (12, 16, 3), 2),
            ((1, 3, 1), (5, 7, 1), 2),
            ((9, 11, 1), (13, 15, 1), 2),
            ((0, 8, 7), (8, 16, 7), 2),
            ((2, 4, 1), (4, 6, 1), 2),
            ((10, 12, 1), (12, 14, 1), 2),
            ((1, 6, 2), (2, 7, 2), 3),
            ((9, 14, 2), (10, 15, 2), 3),
            ((1, 7, 1), (9, 15, 1), 6),
            ((4, 8, 1), (8, 12, 1), 4),
            ((2, 4, 1), (4, 6, 1), 2),
            ((6, 8, 1), (8, 10, 1), 2),
            ((10, 12, 1), (12, 14, 1), 2),
            ((1, 14, 2), (2, 15, 2), 7),
        ]

        p = pks.ap()
        mt = mtA.ap()
        for (a0, a1, ast), (b0, b1, bst), w in GROUPS:
            A = p[:, :, a0:a1:ast]
            Bp = p[:, :, b0:b1:bst]
            tt(vec, mt[:, :, 0:w], A, Bp, Alu.min)
            tt(vec, Bp, A, Bp, Alu.max)
            vec.tensor_copy(out=A, in_=mt[:, :, 0:w])

        # ---- P4: unpack ----
        act.activation(out=sshift.ap(), in_=pks.ap()[:, 0:128, :],
                       func=Act.Identity, bias=bunA.ap(), scale=1.0 / 129.0)
        act.activation(out=suc.ap(), in_=pks.ap(), func=Act.Identity,
                       bias=bunB.ap(), scale=1.0 / 129.0)
        # e-128 = p - 129*s (disk rows)
        vec.scalar_tensor_tensor(out=etld.ap(), in0=suc.ap()[:, 0:128, :],
                                 scalar=-129.0, in1=pks.ap()[:, 0:128, :],
                                 op0=Alu.mult, op1=Alu.add)

        # ---- P5: prefix-max ends -> runs -> disk area ----
        tt(vec, ebuf.ap()[:, :, 1:16], etld.ap()[:, :, 1:16], etld.ap()[:, :, 0:15], Alu.max)
        vec.tensor_copy(out=ebuf.ap()[:, :, 0:1], in_=etld.ap()[:, :, 0:1])
        tt(vec, etld.ap()[:, :, 2:16], ebuf.ap()[:, :, 2:16], ebuf.ap()[:, :, 0:14], Alu.max)
        vec.tensor_copy(out=etld.ap()[:, :, 0:2], in_=ebuf.ap()[:, :, 0:2])
        tt(vec, ebuf.ap()[:, :, 4:16], etld.ap()[:, :, 4:16], etld.ap()[:, :, 0:12], Alu.max)
        vec.tensor_copy(out=ebuf.ap()[:, :, 0:4], in_=etld.ap()[:, :, 0:4])
        tt(vec, etld.ap()[:, :, 8:16], ebuf.ap()[:, :, 8:16], ebuf.ap()[:, :, 0:8], Alu.max)
        vec.tensor_copy(out=etld.ap()[:, :, 0:8], in_=ebuf.ap()[:, :, 0:8])
        # u' = min(Rp_j, s'_{j+1});  darea += relu(u' - s'_j)
        tt(vec, uu.ap(), etld.ap()[:, :, 0:D], sshift.ap()[:, :, 1:16], Alu.min)
        tt(vec, dd.ap(), uu.ap(), sshift.ap()[:, :, 0:D], Alu.subtract)
        ts(vec, dd.ap(), dd.ap(), 0.0, Alu.max, 0.0, Alu.add,
           accum=stats.ap()[:, 0:1])

        # ---- P6: intersection over rows 32:96 (int16, DVE) ----
        aAP = suc.ap()[:, 128:192, 0:16:2]
        bAP = suc.ap()[:, 128:192, 1:16:2]
        sp = sshift.ap()[:, 32:96, 0:D].unsqueeze(2).to_broadcast([K, 64, 8, D])
        up = uu.ap()[:, 32:96, :].unsqueeze(2).to_broadcast([K, 64, 8, D])
        ap_ = aAP.unsqueeze(3).to_broadcast([K, 64, 8, D])
        bp_ = bAP.unsqueeze(3).to_broadcast([K, 64, 8, D])
        tt(vec, mx.ap(), sp, ap_, Alu.max)
        tt(vec, mn.ap(), up, bp_, Alu.min)
        tt(vec, df.ap(), mn.ap(), mx.ap(), Alu.subtract)
        ts(vec, df.ap(), df.ap(), 0.0, Alu.max, 0.0, Alu.add,
           accum=stats.ap()[:, 1:2])

        # ---- P6b: poly area = sum(b - a) ----
        tt(vec, dd.ap()[:, 0:64, 0:8], bAP, aAP, Alu.subtract)
        ts(vec, dd.ap()[:, 0:64, 0:8], dd.ap()[:, 0:64, 0:8], 0.0, Alu.add,
           0.0, Alu.add, accum=stats.ap()[:, 3:4])

        # ---- P7: epilogue ----
        itr = stats.ap()[:, 4:5]; uni = stats.ap()[:, 5:6]
        den = stats.ap()[:, 6:7]; pob = stats.ap()[:, 7:8]
        ts(vec, itr, stats.ap()[:, 1:2], 0.0, Alu.add)
        tt(vec, uni, stats.ap()[:, 0:1], stats.ap()[:, 3:4], Alu.add)
        tt(vec, uni, uni, itr, Alu.subtract)
        ts(vec, den, uni, 1e-6, Alu.add)
        vec.reciprocal(out=den, in_=den)
        tt(vec, pob, itr, den, Alu.mult)
        ts(vec, pob, pob, -1.0, Alu.mult, 1.0, Alu.add)      # 1 - inter/union
        tt(vec, colq.ap()[:, 0:1], pob, maskf.ap(), Alu.mult)
        vec.tensor_copy(out=colq.ap()[:, 1:2], in_=maskf.ap())
        nc.tensor.matmul(out=psum.ap(), lhsT=onesv.ap(), rhs=colq.ap(),
                         start=True, stop=True)
        vec.tensor_copy(out=outsb.ap(), in_=psum.ap())
        nc.sync.dma_start(out_d.ap().unsqueeze(0), outsb.ap())

    nc.compile()
    return nc


def _get_nc():
    if "nc" not in _CACHE:
        _CACHE["nc"] = _build_nc()
    return _CACHE["nc"]


def kernel(output, mask, ind, target, freq_mask=None):
    nc = _get_nc()
    from concourse.bass_utils import run_bass_kernel_spmd

    output = np.asarray(output, dtype=np.float32)
    target = np.asarray(target, dtype=np.float32)
    in_maps = []
    for b in range(B):
        in_maps.append({
            "featT": np.ascontiguousarray(output[b].reshape(C, H * W).T),
            "ind": np.asarray(ind[b], dtype=np.int32),
            "target": np.ascontiguousarray(target[b]),
            "mask": np.asarray(mask[b], dtype=np.int32),
        })
    res = run_bass_kernel_spmd(nc, in_maps, core_ids=list(range(B)))
    parts = np.stack([np.asarray(r["out"], dtype=np.float64) for r in res.results])
    loss = parts[:, 0].sum() / (parts[:, 1].sum() + 1e-6)
    return np.float32(loss), np.float32(0.0)


# revision 15
# speedup vs baseline: 2.7957x; 1.2525x over previous
"""DiskLoss Trainium2 kernel (interval-union formulation).

Computes the reference loss:
  pred = gather(output, ind)            # [K,33] per batch
  gt_m = even-odd rasterization of the 16-gon from target   (per object)
  dk_m = union of 15 disks (radius ceil(|pred[:,32]|)) from pred
  per_obj = 1 - inter/(union+1e-6);  loss = sum(m*per_obj)/(sum(m)+1e-6)

Sharding: data-parallel over batch B=8 -> one batch element per NeuronCore.
Each core reduces its 128 objects (object-per-partition layout) to
(sum m*per_obj, sum m); host adds the 8 partial pairs.

Device algorithm — both masks are per-row interval unions, no pixel raster:
  - disks: per (k,y,d) the row span is [s,e) with h=sqrt(relu(r^2-(y-cy)^2)),
    s=max(0,floor(cx-h)+1), e=min(128,floor(cx+h)+1); floor/clip via ACT
    round-to-nearest int conversion (+-0.5 bias) with uint8 saturation;
    pack p=s*129+(e-128) int16, Batcher-sort the 16 slots per row, unpack
    via ACT, prefix-max ends -> disjoint runs [s'_j, u'_j) (values shifted
    by -128); disk area = sum relu(u'-s')
  - polygon: crossings xint per (y in 32:96, v) in fp32; straddle via
    (y-y1)(y-y2)<0; c-128 = round(xint-95.5) via ACT, zeroed (sentinel)
    unless straddle, packed *130 and sorted in the same Batcher pass
    (rows 128:192); even-odd pairs (a_i,b_i) of the sorted crossings are
    the disjoint poly intervals; poly area = sum(b-a)
  - intersection = sum_{i,j} relu(min(b_i,u'_j) - max(a_i,s'_j)) over the
    poly-overlap rows 32:96 (8 poly pairs x 15 disk runs, int16)
  - DVE does min/max/sort; Pool (gpsimd) does arithmetic (add/sub/mult
    only on this ISA); ACT does all float->int rounding; PE does the final
    masked reduction via ones-matmul over partitions
"""

import sys

if "/opt/trn_rl_repo" not in sys.path:
    sys.path.insert(0, "/opt/trn_rl_repo")

import numpy as np

B, C, H, W = 8, 33, 128, 128
K = 128
V = 16          # polygon vertices
D = 15          # disk centers

_CACHE = {}


def _build_nc():
    import concourse.bacc as bacc
    import concourse.mybir as mybir
    import concourse.tile as tile
    import concourse.bass as bass

    F32 = mybir.dt.float32
    F16 = mybir.dt.float16
    I32 = mybir.dt.int32
    I16 = mybir.dt.int16
    U8 = mybir.dt.uint8
    Alu = mybir.AluOpType
    Act = mybir.ActivationFunctionType

    nc = bacc.Bacc("TRN2", target_bir_lowering=False, debug=False)

    # ---- DRAM I/O (per core) ----
    featT_d = nc.dram_tensor("featT", [H * W, C], F32, kind="ExternalInput")
    ind_d = nc.dram_tensor("ind", [K], I32, kind="ExternalInput")
    tgt_d = nc.dram_tensor("target", [K, C], F32, kind="ExternalInput")
    mask_d = nc.dram_tensor("mask", [K], I32, kind="ExternalInput")
    out_d = nc.dram_tensor("out", [2], F32, kind="ExternalOutput")

    # ---- SBUF ----
    pred = nc.alloc_sbuf_tensor("pred", [K, C], F32)
    tgt = nc.alloc_sbuf_tensor("tgt", [K, C], F32)
    indc = nc.alloc_sbuf_tensor("indc", [K, 1], I32)
    maski = nc.alloc_sbuf_tensor("maski", [K, 1], I32)
    maskf = nc.alloc_sbuf_tensor("maskf", [K, 1], F32)

    pxi = nc.alloc_sbuf_tensor("pxi", [K, W], I32)
    pyg = nc.alloc_sbuf_tensor("pyg", [K, W], F32)      # y global 0..127
    pysh = nc.alloc_sbuf_tensor("pysh", [K, 64], F32)   # y-32 for rows 32:96

    # disk geometry
    negcu = nc.alloc_sbuf_tensor("negcu", [K, D], F32)  # -(cy+32)
    cxg = nc.alloc_sbuf_tensor("cxg", [K, D], F32)      # cx+32
    rsc = nc.alloc_sbuf_tensor("rsc", [K, 4], F32)
    ri = nc.alloc_sbuf_tensor("ri", [K, 1], I32)
    r2u = nc.alloc_sbuf_tensor("r2u", [K, 1], F32)
    sqyu = nc.alloc_sbuf_tensor("sqyu", [K, H, D], F32)
    hsq = nc.alloc_sbuf_tensor("hsq", [K, H, D], F32)
    hh = nc.alloc_sbuf_tensor("hh", [K, H, D], F32)
    lo = nc.alloc_sbuf_tensor("lo", [K, H, D], F32)
    hi = nc.alloc_sbuf_tensor("hi", [K, H, D], F32)
    s8 = nc.alloc_sbuf_tensor("s8", [K, H, D], U8)
    e8 = nc.alloc_sbuf_tensor("e8", [K, H, D], U8)

    # polygon geometry
    x2b = nc.alloc_sbuf_tensor("x2b", [K, V], F32)
    y2b = nc.alloc_sbuf_tensor("y2b", [K, V], F32)
    pv1 = nc.alloc_sbuf_tensor("pv1", [K, V], F32)
    pv2 = nc.alloc_sbuf_tensor("pv2", [K, V], F32)
    pv3 = nc.alloc_sbuf_tensor("pv3", [K, V], F32)
    xa = nc.alloc_sbuf_tensor("xa", [K, 64, V], F32)
    ya = nc.alloc_sbuf_tensor("ya", [K, 64, V], F32)
    xb = nc.alloc_sbuf_tensor("xb", [K, 64, V], F32)
    nei = nc.alloc_sbuf_tensor("nei", [K, 64, V], I16)
    cpre = nc.alloc_sbuf_tensor("cpre", [K, 64, V], I16)

    # sort + runs (all int16)
    pks = nc.alloc_sbuf_tensor("pks", [K, 192, 16], I16)
    mtA = nc.alloc_sbuf_tensor("mtA", [K, 192, 8], I16)
    sshift = nc.alloc_sbuf_tensor("sshift", [K, H, 16], I16)   # s-128 (disk)
    suc = nc.alloc_sbuf_tensor("suc", [K, 192, 16], I16)       # s | c-128
    etld = nc.alloc_sbuf_tensor("etld", [K, H, 16], I16)       # e-128 sorted
    ebuf = nc.alloc_sbuf_tensor("ebuf", [K, H, 16], I16)
    uu = nc.alloc_sbuf_tensor("uu", [K, H, D], I16)
    dd = nc.alloc_sbuf_tensor("dd", [K, H, D], I16)

    # intersection
    mx = nc.alloc_sbuf_tensor("mx", [K, 64, 8, D], I16)
    mn = nc.alloc_sbuf_tensor("mn", [K, 64, 8, D], I16)
    df = nc.alloc_sbuf_tensor("df", [K, 64, 8, D], I16)
    aE = nc.alloc_sbuf_tensor("aE", [K, 64, 8, D], I16)
    bE = nc.alloc_sbuf_tensor("bE", [K, 64, 8, D], I16)

    # act bias constants
    bm95 = nc.alloc_sbuf_tensor("bm95", [K, 1], F32)     # -95.5 (poly c shift)
    bp05 = nc.alloc_sbuf_tensor("bp05", [K, 1], F32)     # +0.5
    b1275 = nc.alloc_sbuf_tensor("b1275", [K, 1], F32)   # +127.5

    # reduction
    stats = nc.alloc_sbuf_tensor("stats", [K, 8], F32)
    onesv = nc.alloc_sbuf_tensor("onesv", [K, 1], F32)
    colq = nc.alloc_sbuf_tensor("colq", [K, 2], F32)
    outsb = nc.alloc_sbuf_tensor("outsb", [1, 2], F32)
    psum = nc.alloc_psum_tensor("psum", [1, 2], F32)

    with tile.TileContext(nc) as tc:
        vec = nc.vector
        gps = nc.gpsimd
        act = nc.scalar

        def ts(eng, out, in0, s1, op0, s2=None, op1=None, accum=None):
            kw = {}
            if accum is not None:
                kw["accum_out"] = accum
            if op1 is not None:
                return eng.tensor_scalar(out=out, in0=in0, scalar1=s1, scalar2=s2,
                                         op0=op0, op1=op1, **kw)
            return eng.tensor_scalar(out=out, in0=in0, scalar1=s1, scalar2=None,
                                     op0=op0, **kw)

        def tt(eng, out, in0, in1, op):
            return eng.tensor_tensor(out=out, in0=in0, in1=in1, op=op)

        # ---- P0: input DMAs + gather + iotas + consts ----
        nc.sync.dma_start(indc.ap(), ind_d.ap().unsqueeze(1))
        nc.sync.dma_start(tgt.ap(), tgt_d.ap())
        nc.sync.dma_start(maski.ap(), mask_d.ap().unsqueeze(1))
        nc.gpsimd.indirect_dma_start(
            out=pred.ap(), out_offset=None, in_=featT_d.ap(),
            in_offset=bass.IndirectOffsetOnAxis(ap=indc.ap(), axis=0))

        nc.gpsimd.iota(pxi.ap(), pattern=[[1, W]], base=0, channel_multiplier=0)
        ts(vec, pyg.ap(), pxi.ap(), 0.0, Alu.add)            # int->f32, 0..127
        ts(vec, pysh.ap(), pxi.ap()[:, 32:96], -32.0, Alu.add)
        ts(vec, maskf.ap(), maski.ap(), 0.0, Alu.add)
        vec.memset(bm95.ap(), -95.5)
        vec.memset(bp05.ap(), 0.5)
        vec.memset(b1275.ap(), 127.5)
        vec.memset(onesv.ap(), 1.0)

        # ---- P2a: disk scalars first (unblocks the ACT Square chain) ----
        ts(vec, negcu.ap(), pred.ap()[:, 1:2 * D:2], -1.0, Alu.mult, -32.0, Alu.add)
        ts(vec, cxg.ap(), pred.ap()[:, 0:2 * D:2], 32.0, Alu.add)
        u = rsc.ap()[:, 0:1]; t = rsc.ap()[:, 1:2]; g = rsc.ap()[:, 2:3]
        ts(vec, t, pred.ap()[:, 32:33], -1.0, Alu.mult)
        tt(vec, u, pred.ap()[:, 32:33], t, Alu.max)          # |p|
        vec.tensor_copy(out=ri.ap(), in_=u)
        vec.tensor_copy(out=t, in_=ri.ap())
        tt(vec, g, t, u, Alu.is_gt)
        tt(vec, t, t, g, Alu.subtract)                       # floor
        tt(vec, g, u, t, Alu.is_gt)
        tt(vec, t, t, g, Alu.add)                            # ceil = r
        tt(vec, r2u.ap(), t, t, Alu.mult)                    # r^2

        for d in range(D):
            act.activation(out=sqyu.ap()[:, :, d], in_=pyg.ap(), func=Act.Square,
                           bias=negcu.ap()[:, d:d + 1], scale=1.0)

        # ---- P1: polygon precompute (raw coords; rows are y-32 in 0..63) ----
        x1v = tgt.ap()[:, 0:2 * V:2]
        y1v = tgt.ap()[:, 1:2 * V:2]
        gps.tensor_copy(out=x2b.ap()[:, 0:V - 1], in_=tgt.ap()[:, 2:2 * V:2])
        gps.tensor_copy(out=x2b.ap()[:, V - 1:V], in_=tgt.ap()[:, 0:1])
        gps.tensor_copy(out=y2b.ap()[:, 0:V - 1], in_=tgt.ap()[:, 3:2 * V:2])
        gps.tensor_copy(out=y2b.ap()[:, V - 1:V], in_=tgt.ap()[:, 1:2])
        d0 = pv1.ap(); eqz = pv2.ap(); sl = pv3.ap()
        tt(vec, d0, y2b.ap(), y1v, Alu.subtract)
        ts(vec, eqz, d0, 0.0, Alu.is_equal)
        tt(vec, d0, d0, eqz, Alu.add)                        # denom
        vec.reciprocal(out=eqz, in_=d0)                      # 1/denom
        tt(vec, sl, x2b.ap(), x1v, Alu.subtract)
        tt(vec, sl, sl, eqz, Alu.mult)                       # slope

        pyb = pysh.ap().unsqueeze(2).to_broadcast([K, 64, V])
        y1b = y1v.unsqueeze(1).to_broadcast([K, 64, V])
        y2bb = y2b.ap().unsqueeze(1).to_broadcast([K, 64, V])
        # straddle = (y-y1)(y-y2) < 0  (a.s. equal to reference predicate)
        tt(vec, xa.ap(), pyb, y1b, Alu.subtract)             # y-y1 (also xint)
        tt(gps, ya.ap(), pyb, y2bb, Alu.subtract)            # y-y2
        tt(gps, ya.ap(), ya.ap(), xa.ap(), Alu.mult)
        ts(vec, nei.ap(), ya.ap(), 0.0, Alu.is_lt)           # straddle 0/1 i16
        # xint (raw coords) = x1 + (y - y1)*slope
        tt(vec, xb.ap(), xa.ap(), sl.unsqueeze(1).to_broadcast([K, 64, V]), Alu.mult)
        tt(gps, xa.ap(), xb.ap(), x1v.unsqueeze(1).to_broadcast([K, 64, V]), Alu.add)
        # c-128 = round(xint_raw - 95.5); garbage for non-straddle (zeroed next)
        act.activation(out=cpre.ap(), in_=xa.ap(), func=Act.Identity,
                       bias=bm95.ap(), scale=1.0)
        # pack poly rows: p = (c-128)*130 * straddle
        vec.scalar_tensor_tensor(out=pks.ap()[:, 128:192, :], in0=cpre.ap(),
                                 scalar=130.0, in1=nei.ap(),
                                 op0=Alu.mult, op1=Alu.mult)

        # ---- P2b: disk per-row geometry ----
        # hsqn = min(sqyu - r^2, 0);  h = sqrt(-hsqn)
        ts(vec, hsq.ap(), sqyu.ap(), r2u.ap(), Alu.subtract, 0.0, Alu.min)
        act.activation(out=hh.ap(), in_=hsq.ap(), func=Act.Sqrt,
                       bias=0.0, scale=-1.0)
        cxb = cxg.ap().unsqueeze(1).to_broadcast([K, H, D])
        tt(gps, lo.ap(), cxb, hh.ap(), Alu.subtract)
        tt(vec, hi.ap(), cxb, hh.ap(), Alu.add)
        # s = round(lo+0.5) sat to [0,255];  e_rev = round(127.5-hi) sat
        act.activation(out=s8.ap(), in_=lo.ap(), func=Act.Identity,
                       bias=bp05.ap(), scale=1.0)
        act.activation(out=e8.ap(), in_=hi.ap(), func=Act.Identity,
                       bias=b1275.ap(), scale=-1.0)
        gps.memset(pks.ap()[:, 0:128, D:16], 16512)          # s=128,e=128 sentinel

        # ---- P3: Batcher odd-even mergesort of the 16 slots (all rows) ----
        GROUPS = [
            ((0, 15, 2), (1, 16, 2), 8),
            ((0, 2, 1), (2, 4, 1), 2),
            ((4, 6, 1), (6, 8, 1), 2),
            ((8, 10, 1), (10, 12, 1), 2),
            ((12, 14, 1), (14, 16, 1), 2),
            ((1, 14, 4), (2, 15, 4), 4),
            ((0, 4, 3), (4, 8, 3), 2),
            ((8, 12, 3), (12, 16, 3), 2),
            ((1, 3, 1), (5, 7, 1), 2),
            ((9, 11, 1), (13, 15, 1), 2),
            ((0, 8, 7), (8, 16, 7), 2),
            ((2, 4, 1), (4, 6, 1), 2),
            ((10, 12, 1), (12, 14, 1), 2),
            ((1, 6, 2), (2, 7, 2), 3),
            ((9, 14, 2), (10, 15, 2), 3),
            ((1, 7, 1), (9, 15, 1), 6),
            ((4, 8, 1), (8, 12, 1), 4),
            ((2, 4, 1), (4, 6, 1), 2),
            ((6, 8, 1), (8, 10, 1), 2),
            ((10, 12, 1), (12, 14, 1), 2),
            ((1, 14, 2), (2, 15, 2), 7),
        ]

        def emit_sort(r0, r1):
            p = pks.ap()[:, r0:r1, :]
            mt = mtA.ap()[:, r0:r1, :]
            for (a0, a1, ast), (b0, b1, bst), w in GROUPS:
                A = p[:, :, a0:a1:ast]
                Bp = p[:, :, b0:b1:bst]
                tt(vec, mt[:, :, 0:w], A, Bp, Alu.min)
                tt(vec, Bp, A, Bp, Alu.max)
                vec.tensor_copy(out=A, in_=mt[:, :, 0:w])

        # poly rows sort first (ready ~15us before the disk rows), then the
        # poly-side unpack + operand materialization overlap the disk sort
        emit_sort(128, 192)
        ts(vec, suc.ap()[:, 128:192, :], pks.ap()[:, 128:192, :],
           1.0 / 129.0, Alu.mult, 128.0 / 129.0 - 0.5, Alu.add)
        aAP = suc.ap()[:, 128:192, 0:16:2]
        bAP = suc.ap()[:, 128:192, 1:16:2]
        # poly area = sum(b - a) (early; dd rows reused later is fine)
        tt(vec, dd.ap()[:, 0:64, 0:8], bAP, aAP, Alu.subtract)
        ts(vec, dd.ap()[:, 0:64, 0:8], dd.ap()[:, 0:64, 0:8], 0.0, Alu.add,
           0.0, Alu.add, accum=stats.ap()[:, 3:4])
        # materialize a/b over the disk-run axis on ACT (overlaps disk sort)
        act.activation(out=aE.ap(), func=Act.Identity, bias=0.0, scale=1.0,
                       in_=aAP.unsqueeze(3).to_broadcast([K, 64, 8, D]))
        act.activation(out=bE.ap(), func=Act.Identity, bias=0.0, scale=1.0,
                       in_=bAP.unsqueeze(3).to_broadcast([K, 64, 8, D]))

        # pack disk rows: p = s*129 - e_rev = s*129 + (e-128)
        vec.scalar_tensor_tensor(out=pks.ap()[:, 0:128, 0:D], in0=s8.ap(),
                                 scalar=129.0, in1=e8.ap(),
                                 op0=Alu.mult, op1=Alu.subtract)
        emit_sort(0, 128)

        # ---- P4: unpack (DVE ts computes in fp32, rounds on int16 out) ----
        ts(vec, sshift.ap(), pks.ap()[:, 0:128, :], 1.0 / 129.0, Alu.mult,
           128.0 / 129.0 - 0.5 - 128.0, Alu.add)
        ts(vec, suc.ap()[:, 0:128, :], pks.ap()[:, 0:128, :], 1.0 / 129.0,
           Alu.mult, 128.0 / 129.0 - 0.5, Alu.add)
        # e-128 = p - 129*s (disk rows)
        vec.scalar_tensor_tensor(out=etld.ap(), in0=suc.ap()[:, 0:128, :],
                                 scalar=-129.0, in1=pks.ap()[:, 0:128, :],
                                 op0=Alu.mult, op1=Alu.add)

        # ---- P5: prefix-max ends -> runs -> disk area ----
        tt(vec, ebuf.ap()[:, :, 1:16], etld.ap()[:, :, 1:16], etld.ap()[:, :, 0:15], Alu.max)
        vec.tensor_copy(out=ebuf.ap()[:, :, 0:1], in_=etld.ap()[:, :, 0:1])
        tt(vec, etld.ap()[:, :, 2:16], ebuf.ap()[:, :, 2:16], ebuf.ap()[:, :, 0:14], Alu.max)
        vec.tensor_copy(out=etld.ap()[:, :, 0:2], in_=ebuf.ap()[:, :, 0:2])
        tt(vec, ebuf.ap()[:, :, 4:16], etld.ap()[:, :, 4:16], etld.ap()[:, :, 0:12], Alu.max)
        vec.tensor_copy(out=ebuf.ap()[:, :, 0:4], in_=etld.ap()[:, :, 0:4])
        tt(vec, etld.ap()[:, :, 8:16], ebuf.ap()[:, :, 8:16], ebuf.ap()[:, :, 0:8], Alu.max)
        vec.tensor_copy(out=etld.ap()[:, :, 0:8], in_=ebuf.ap()[:, :, 0:8])
        # u' = min(Rp_j, s'_{j+1});  darea += relu(u' - s'_j)
        tt(vec, uu.ap(), etld.ap()[:, :, 0:D], sshift.ap()[:, :, 1:16], Alu.min)
        tt(vec, dd.ap(), uu.ap(), sshift.ap()[:, :, 0:D], Alu.subtract)
        ts(vec, dd.ap(), dd.ap(), 0.0, Alu.max, 0.0, Alu.add,
           accum=stats.ap()[:, 0:1])

        # ---- P6: intersection over rows 32:96 (int16, DVE, 2x mode) ----
        sp = sshift.ap()[:, 32:96, 0:D].unsqueeze(2).to_broadcast([K, 64, 8, D])
        up = uu.ap()[:, 32:96, :].unsqueeze(2).to_broadcast([K, 64, 8, D])
        tt(vec, mx.ap(), sp, aE.ap(), Alu.max)
        tt(vec, mn.ap(), up, bE.ap(), Alu.min)
        tt(vec, df.ap(), mn.ap(), mx.ap(), Alu.subtract)
        ts(vec, df.ap(), df.ap(), 0.0, Alu.max, 0.0, Alu.add,
           accum=stats.ap()[:, 1:2])

        # ---- P7: epilogue ----
        itr = stats.ap()[:, 4:5]; uni = stats.ap()[:, 5:6]
        den = stats.ap()[:, 6:7]; pob = stats.ap()[:, 7:8]
        ts(vec, itr, stats.ap()[:, 1:2], 0.0, Alu.add)
        tt(vec, uni, stats.ap()[:, 0:1], stats.ap()[:, 3:4], Alu.add)
        tt(vec, uni, uni, itr, Alu.subtract)
        ts(vec, den, uni, 1e-6, Alu.add)
        vec.reciprocal(out=den, in_=den)
        tt(vec, pob, itr, den, Alu.mult)
        ts(vec, pob, pob, -1.0, Alu.mult, 1.0, Alu.add)      # 1 - inter/union
        tt(vec, colq.ap()[:, 0:1], pob, maskf.ap(), Alu.mult)
        vec.tensor_copy(out=colq.ap()[:, 1:2], in_=maskf.ap())
        nc.tensor.matmul(out=psum.ap(), lhsT=onesv.ap(), rhs=colq.ap(),
                         start=True, stop=True)
        vec.tensor_copy(out=outsb.ap(), in_=psum.ap())
        nc.sync.dma_start(out_d.ap().unsqueeze(0), outsb.ap())

    nc.compile()
    return nc


def _get_nc():
    if "nc" not in _CACHE:
        _CACHE["nc"] = _build_nc()
    return _CACHE["nc"]


def kernel(output, mask, ind, target, freq_mask=None):
    nc = _get_nc()
    from concourse.bass_utils import run_bass_kernel_spmd

    output = np.asarray(output, dtype=np.float32)
    target = np.asarray(target, dtype=np.float32)
    in_maps = []
    for b in range(B):
        in_maps.append({
            "featT": np.ascontiguousarray(output[b].reshape(C, H * W).T),
            "ind": np.asarray(ind[b], dtype=np.int32),
            "target": np.ascontiguousarray(target[b]),
            "mask": np.asarray(mask[b], dtype=np.int32),
        })
    res = run_bass_kernel_spmd(nc, in_maps, core_ids=list(range(B)))
    parts = np.stack([np.asarray(r["out"], dtype=np.float64) for r in res.results])
    loss = parts[:, 0].sum() / (parts[:, 1].sum() + 1e-6)
    return np.float32(loss), np.float32(0.0)
